# revision 21
# baseline (speedup 1.0000x reference)
"""ConvHex GNN message-passing kernel for Trainium2 (8 NeuronCores).

Math (per batch b):
    out[b,o,h] = ( Wc[o,:] @ x[b,:,h]
                   + sum_k Wn[o,:,k] @ x[b,:,idx[h,k]]*valid ) / nu + bias[o]

Strategy (V6):
  - Hybrid shard: batch x4, H x2 -> 8 cores.  64 batches + ~928 dest
    pixels per core.
  - x is quantized host-side to fp8 e3m4 (TRN float8e3, 4-bit mantissa:
    ~3% rel err on N(0,1) data -> end-to-end ~1.4e-2, under the 2e-2
    gate).  This HALVES the dominant gather DMA volume vs fp16.
  - Neighbor gather via SWDGE dma_gather(transpose=True) from an
    HBM-resident token table.  Tokens are 4KB = 64 batches x 64 ch fp8,
    custom byte order so the u16-granularity X-bar transpose lands
    compute layout: u16 j = q*128 + (b%2)*64 + c holds the fp8 pair
    (x[4q+(b%2)... m=0], x[4q+2+... m=1]); partition p=(b%2)*64+c, free
    (q, idx, m).  Matmul moving AP bitcast to fp8e3 [p, q, m, idx] ->
    column order = batch-pair-major, matching PSUM [p, 4, LIVE].
  - fp16 block-diag [[W.T,0],[0,W.T]] stationary weights (scaled 1/nu),
    mixed fp16xfp8 matmul at full rate; 7 PSUM-accumulated matmuls per
    (chunk, quad-pair-group), tap-outer order so each tap's matmuls
    only wait on that tap's gather.
  - Center tap via direct chunked fp8 DMA.  Invalid neighbors -> zero
    row at H.  Output fp16, staged per 4-chunk group, one big store.
"""

import numpy as np
import ml_dtypes

import concourse.bacc as bacc
import concourse.mybir as mybir
import concourse.tile as tile
from concourse import bass_utils

B, C, H, K = 256, 64, 1855, 6
NCORES = 8
NB = 4                    # batch blocks
NH = 2                    # h halves
BL = B // NB              # 64 batches per core
NPAIR = BL // 2           # 32
NQUAD = BL // 4           # 16 quad-batch token slots
HP = H + 1                # zero row at H
P = 128
ELEM = BL * C // 2        # 2048 u16 per token (4KB fp8 payload)
NI = 128                  # static idxs per gather call
LIVE = 116                # live idxs per call (ring: 116+2=118<128)
NCHUNK = 8                # chunks per h-half
HHALF = NCHUNK * LIVE     # 928 pixels per half
H0 = [0, 927]             # half start (pixel 927 computed by both halves)
HLEN = [928, 928]         # live pixels per half
GRP = 2                   # chunks per store group
NGRP = NCHUNK // GRP      # 4
GW = GRP * LIVE           # 232 pixels per group
NQ = 4

_F32 = mybir.dt.float32
_F16 = mybir.dt.float16
_F8 = mybir.dt.float8e3
_I16 = mybir.dt.int16
_E3M4 = ml_dtypes.float8_e3m4


def _host_prep(x, neighbors, weight_center, weight_neighbors, bias):
    x = np.asarray(x, dtype=np.float32)
    neighbors = np.asarray(neighbors)
    wc = np.asarray(weight_center, dtype=np.float32)
    wn = np.asarray(weight_neighbors, dtype=np.float32)
    bias = np.asarray(bias, dtype=np.float32)

    nu = np.float32((neighbors[0] >= 0).sum() + 1)
    # invalid neighbors and pad slots both hit the zero row at H
    safe = np.where(neighbors >= 0, neighbors, H).astype(np.int16)  # [H,K]

    x8 = np.clip(x, -15.5, 15.5).astype(_E3M4).view(np.uint8)  # [B, C, H]

    # token tables per batch-block: byte 2*(q*128+bp*64+c)+m of token h
    # = x8[bi*64 + 4q + 2m + bp, c, h]
    xtab = np.zeros((NB, HP, 2 * ELEM), dtype=np.uint8)
    for bi in range(NB):
        xb = x8[bi * BL:(bi + 1) * BL]                  # [64, C, H]
        t = xb.reshape(NQUAD, 2, 2, C, H)               # [q, m, bp, c, h]
        t = t.transpose(4, 0, 2, 3, 1)                  # [h, q, bp, c, m]
        xtab[bi, :H] = t.reshape(H, 2 * ELEM)
    xtab = xtab.view(np.float16)                        # [NB, HP, ELEM]

    # index tables per h-half: idx_w[hj] = [128, K*NCHUNK*(NI//16)] int16
    # pad slots are -1 (never gathered: num_idxs_reg counts non-negatives)
    idx_w = []
    for hj in range(NH):
        idx_pack = np.full((K, NCHUNK, NI), -1, dtype=np.int16)
        for k in range(K):
            for ci in range(NCHUNK):
                c0 = H0[hj] + ci * LIVE
                idx_pack[k, ci, :LIVE] = safe[c0:c0 + LIVE, k]
        w = idx_pack.reshape(K, NCHUNK, NI // 16, 16)
        iw = np.tile(w.transpose(3, 0, 1, 2), (8, 1, 1, 1))
        idx_w.append(np.ascontiguousarray(
            iw.reshape(P, K * NCHUNK * (NI // 16))))

    # center operand, per core (bi, hj), fp8:
    # x_all[core][(b%2)*64+c, ci, pair, j] = x8[bi*64+b, c, H0+ci*116+j]
    x_all = np.zeros((NCORES, P, NCHUNK, NPAIR, NI), dtype=np.uint8)
    for bi in range(NB):
        xp = x8[bi * BL:(bi + 1) * BL].reshape(NPAIR, 2, C, H)
        for hj in range(NH):
            core = bi * NH + hj
            for ci in range(NCHUNK):
                c0 = H0[hj] + ci * LIVE
                blk = xp[:, :, :, c0:c0 + LIVE]          # [pair, bp, c, n]
                blk = blk.transpose(1, 2, 0, 3)          # [bp, c, pair, n]
                x_all[core, :, ci, :, :LIVE] = blk.reshape(P, NPAIR, LIVE)
    x_all = np.ascontiguousarray(x_all).view(_E3M4)

    # fp16 block-diag weights / nu, packed [128, 7*128]
    w_all = np.zeros((K + 1, P, P), dtype=np.float16)
    mats = [wc] + [wn[:, :, k] for k in range(K)]
    for s, wmat in enumerate(mats):
        wt = (wmat.T / nu).astype(np.float16)
        w_all[s, :C, :C] = wt
        w_all[s, C:, C:] = wt
    w_pack = np.ascontiguousarray(
        w_all.transpose(1, 0, 2).reshape(P, (K + 1) * P))

    bias2 = np.concatenate([bias, bias]).reshape(P, 1).astype(np.float32)
    return xtab, x_all, idx_w, w_pack, bias2


def _build_program(w_pack, bias2):
    nc = bacc.Bacc("TRN2", target_bir_lowering=False, debug=False,
                   num_devices=NCORES, num_swdge_queues=NQ,
                   enable_asserts=False)

    xtab_d = nc.dram_tensor("xtab", [HP, ELEM], _F16, kind="ExternalInput")
    xall_d = nc.dram_tensor("xall", [P, NCHUNK, NPAIR, NI], _F8,
                            kind="ExternalInput")
    idx_d = nc.dram_tensor("idxw", [P, K * NCHUNK * (NI // 16)], _I16,
                           kind="ExternalInput")
    out_d = nc.dram_tensor("out", [NGRP, P, NPAIR, GW], _F16,
                           kind="ExternalOutput")

    w_dram = nc.inline_tensor(w_pack, name="w_pack")
    b_dram = nc.inline_tensor(bias2, name="bias2")

    call_no = 0
    with tile.TileContext(nc) as tc:
        with (
            tc.tile_pool(name="consts", bufs=1) as cpool,
            tc.tile_pool(name="gp", bufs=12) as gpool,
            tc.tile_pool(name="op", bufs=2) as opool,
            tc.tile_pool(name="ps", bufs=8, space="PSUM") as pspool,
        ):
            # idx table first: gather descriptor prep only waits on this
            idx_sb = cpool.tile([P, K * NCHUNK * (NI // 16)], _I16)
            nc.sync.dma_start(idx_sb[:], idx_d[:])
            w_sb = cpool.tile([P, K + 1, P], _F16)
            nc.sync.dma_start(w_sb[:], w_dram[:])
            b_sb = cpool.tile([P, 1], _F32)
            nc.sync.dma_start(b_sb[:], b_dram[:])
            # whole center operand resident: no per-chunk load dependency
            x_sb = cpool.tile([P, NCHUNK, NPAIR, NI], _F8)
            nc.sync.dma_start(x_sb[:], xall_d[:])

            for g in range(NGRP):
                o_t = opool.tile([P, NPAIR, GW], _F16, name="o_t", tag="o_t")
                for cl in range(GRP):
                    ci = g * GRP + cl
                    g_ts = []
                    for k in range(K):
                        g_t = gpool.tile([P, NQUAD, NI], _F16)
                        io = (k * NCHUNK + ci) * (NI // 16)
                        nc.gpsimd.dma_gather(
                            g_t[:], xtab_d[:], idx_sb[:, io:io + NI // 16],
                            num_idxs=NI, num_idxs_reg=LIVE,
                            elem_size=ELEM, transpose=True,
                            queue_num=call_no % NQ)
                        call_no += 1
                        # fp8 view [p, q, m, idx]: pair index = 2q + m
                        g_ts.append(g_t[:].bitcast(_F8).rearrange(
                            "p q (i m) -> p q m i", m=2))
                    pss = [pspool.tile([P, 4, LIVE], _F32, name="ps",
                                       tag="ps")
                           for qd in range(NPAIR // 4)]
                    # tap-outer: each tap's matmuls only wait on its gather;
                    # center last (its operand is resident from startup)
                    for k in range(K):
                        for qd in range(NPAIR // 4):
                            nc.tensor.matmul(
                                pss[qd][:, :, :], w_sb[:, k + 1, :],
                                g_ts[k][:, qd * 2:qd * 2 + 2, :, :LIVE],
                                start=(k == 0), stop=False)
                    for qd in range(NPAIR // 4):
                        nc.tensor.matmul(
                            pss[qd][:, :, :], w_sb[:, 0, :],
                            x_sb[:, ci, qd * 4:qd * 4 + 4, :LIVE],
                            start=False, stop=True)
                        nc.vector.tensor_scalar_add(
                            o_t[:, qd * 4:qd * 4 + 4,
                                cl * LIVE:cl * LIVE + LIVE],
                            pss[qd][:, :, :], b_sb[:, :1])
                nc.sync.dma_start(out_d[g], o_t[:])

    nc.compile()
    return nc


def _run(inputs, trace=False):
    xtab, x_all, idx_w, w_pack, bias2 = _host_prep(
        inputs["x"], inputs["neighbors"], inputs["weight_center"],
        inputs["weight_neighbors"], inputs["bias"])
    nc = _build_program(w_pack, bias2)
    in_maps = []
    for bi in range(NB):
        for hj in range(NH):
            core = bi * NH + hj
            in_maps.append({"xtab": xtab[bi], "xall": x_all[core],
                            "idxw": idx_w[hj]})
    res = None
    for attempt in range(3):
        try:
            res = bass_utils.run_bass_kernel_spmd(
                nc, in_maps, core_ids=list(range(NCORES)), trace=trace)
            break
        except Exception:
            # transient NRT/device hiccups: retry (recompiles nothing)
            if attempt == 2:
                raise
    out = np.zeros((B, C, H), dtype=np.float32)
    for bi in range(NB):
        for hj in range(NH):
            core = bi * NH + hj
            r = np.asarray(res.results[core]["out"])  # [NGRP,128,NPAIR,GW]
            r = r.reshape(NGRP, 2, C, NPAIR, GW).astype(np.float32)
            r = r.transpose(3, 1, 2, 0, 4).reshape(BL, C, HHALF)
            out[bi * BL:(bi + 1) * BL, :, H0[hj]:H0[hj] + HHALF] = r
    return np.ascontiguousarray(out), res


def kernel(x, neighbors, weight_center, weight_neighbors, bias):
    out, _ = _run(dict(x=x, neighbors=neighbors, weight_center=weight_center,
                       weight_neighbors=weight_neighbors, bias=bias))
    return out


# revision 23
# speedup vs baseline: 1.0918x; 1.0918x over previous
"""ConvHex GNN message-passing kernel for Trainium2 (8 NeuronCores).

Math (per batch b):
    out[b,o,h] = ( Wc[o,:] @ x[b,:,h]
                   + sum_k Wn[o,:,k] @ x[b,:,idx[h,k]]*valid ) / nu + bias[o]

Strategy (V6):
  - Hybrid shard: batch x4, H x2 -> 8 cores.  64 batches + ~928 dest
    pixels per core.
  - x is quantized host-side to fp8 e3m4 (TRN float8e3, 4-bit mantissa:
    ~3% rel err on N(0,1) data -> end-to-end ~1.4e-2, under the 2e-2
    gate).  This HALVES the dominant gather DMA volume vs fp16.
  - Neighbor gather via SWDGE dma_gather(transpose=True) from an
    HBM-resident token table.  Tokens are 4KB = 64 batches x 64 ch fp8,
    custom byte order so the u16-granularity X-bar transpose lands
    compute layout: u16 j = q*128 + (b%2)*64 + c holds the fp8 pair
    (x[4q+(b%2)... m=0], x[4q+2+... m=1]); partition p=(b%2)*64+c, free
    (q, idx, m).  Matmul moving AP bitcast to fp8e3 [p, q, m, idx] ->
    column order = batch-pair-major, matching PSUM [p, 4, LIVE].
  - fp16 block-diag [[W.T,0],[0,W.T]] stationary weights (scaled 1/nu),
    mixed fp16xfp8 matmul at full rate; 7 PSUM-accumulated matmuls per
    (chunk, quad-pair-group), tap-outer order so each tap's matmuls
    only wait on that tap's gather.
  - Center tap via direct chunked fp8 DMA.  Invalid neighbors -> zero
    row at H.  Output fp16, staged per 4-chunk group, one big store.
"""

import numpy as np
import ml_dtypes

import concourse.bacc as bacc
import concourse.mybir as mybir
import concourse.tile as tile
from concourse import bass_utils

B, C, H, K = 256, 64, 1855, 6
NCORES = 8
NB = 4                    # batch blocks
NH = 2                    # h halves
BL = B // NB              # 64 batches per core
NPAIR = BL // 2           # 32
NQUAD = BL // 4           # 16 quad-batch token slots
HP = H + 1                # zero row at H
P = 128
ELEM = BL * C // 2        # 2048 u16 per token (4KB fp8 payload)
NI = 128                  # static idxs per gather call
LIVE = 116                # live idxs per call (ring: 116+2=118<128)
NCHUNK = 8                # chunks per h-half
HHALF = NCHUNK * LIVE     # 928 pixels per half
H0 = [0, 927]             # half start (pixel 927 computed by both halves)
HLEN = [928, 928]         # live pixels per half
GRP = 4                   # chunks per store group
NGRP = NCHUNK // GRP      # 2
GW = GRP * LIVE           # 464 pixels per group
NQ = 4

_F32 = mybir.dt.float32
_F16 = mybir.dt.float16
_F8 = mybir.dt.float8e3
_I16 = mybir.dt.int16
_E3M4 = ml_dtypes.float8_e3m4


def _host_prep(x, neighbors, weight_center, weight_neighbors, bias):
    x = np.asarray(x, dtype=np.float32)
    neighbors = np.asarray(neighbors)
    wc = np.asarray(weight_center, dtype=np.float32)
    wn = np.asarray(weight_neighbors, dtype=np.float32)
    bias = np.asarray(bias, dtype=np.float32)

    nu = np.float32((neighbors[0] >= 0).sum() + 1)
    # invalid neighbors and pad slots both hit the zero row at H
    safe = np.where(neighbors >= 0, neighbors, H).astype(np.int16)  # [H,K]

    x8 = np.clip(x, -15.5, 15.5).astype(_E3M4).view(np.uint8)  # [B, C, H]

    # token tables per batch-block: byte 2*(q*128+bp*64+c)+m of token h
    # = x8[bi*64 + 4q + 2m + bp, c, h]
    xtab = np.zeros((NB, HP, 2 * ELEM), dtype=np.uint8)
    for bi in range(NB):
        xb = x8[bi * BL:(bi + 1) * BL]                  # [64, C, H]
        t = xb.reshape(NQUAD, 2, 2, C, H)               # [q, m, bp, c, h]
        t = t.transpose(4, 0, 2, 3, 1)                  # [h, q, bp, c, m]
        xtab[bi, :H] = t.reshape(H, 2 * ELEM)
    xtab = xtab.view(np.float16)                        # [NB, HP, ELEM]

    # index tables per h-half: idx_w[hj] = [128, K*NCHUNK*(NI//16)] int16
    # pad slots are -1 (never gathered: num_idxs_reg counts non-negatives)
    idx_w = []
    for hj in range(NH):
        idx_pack = np.full((K, NCHUNK, NI), -1, dtype=np.int16)
        for k in range(K):
            for ci in range(NCHUNK):
                c0 = H0[hj] + ci * LIVE
                idx_pack[k, ci, :LIVE] = safe[c0:c0 + LIVE, k]
        w = idx_pack.reshape(K, NCHUNK, NI // 16, 16)
        iw = np.tile(w.transpose(3, 0, 1, 2), (8, 1, 1, 1))
        idx_w.append(np.ascontiguousarray(
            iw.reshape(P, K * NCHUNK * (NI // 16))))

    # center operand, per core (bi, hj), fp8:
    # x_all[core][(b%2)*64+c, ci, pair, j] = x8[bi*64+b, c, H0+ci*116+j]
    x_all = np.zeros((NCORES, P, NCHUNK, NPAIR, NI), dtype=np.uint8)
    for bi in range(NB):
        xp = x8[bi * BL:(bi + 1) * BL].reshape(NPAIR, 2, C, H)
        for hj in range(NH):
            core = bi * NH + hj
            for ci in range(NCHUNK):
                c0 = H0[hj] + ci * LIVE
                blk = xp[:, :, :, c0:c0 + LIVE]          # [pair, bp, c, n]
                blk = blk.transpose(1, 2, 0, 3)          # [bp, c, pair, n]
                x_all[core, :, ci, :, :LIVE] = blk.reshape(P, NPAIR, LIVE)
    x_all = np.ascontiguousarray(x_all).view(_E3M4)

    # fp16 block-diag weights / nu, packed [128, 7*128]
    w_all = np.zeros((K + 1, P, P), dtype=np.float16)
    mats = [wc] + [wn[:, :, k] for k in range(K)]
    for s, wmat in enumerate(mats):
        wt = (wmat.T / nu).astype(np.float16)
        w_all[s, :C, :C] = wt
        w_all[s, C:, C:] = wt
    w_pack = np.ascontiguousarray(
        w_all.transpose(1, 0, 2).reshape(P, (K + 1) * P))

    bias2 = np.concatenate([bias, bias]).reshape(P, 1).astype(np.float32)
    return xtab, x_all, idx_w, w_pack, bias2


def _build_program(w_pack, bias2):
    nc = bacc.Bacc("TRN2", target_bir_lowering=False, debug=False,
                   num_devices=NCORES, num_swdge_queues=NQ,
                   enable_asserts=False)

    xtab_d = nc.dram_tensor("xtab", [HP, ELEM], _F16, kind="ExternalInput")
    xall_d = nc.dram_tensor("xall", [P, NCHUNK, NPAIR, NI], _F8,
                            kind="ExternalInput")
    idx_d = nc.dram_tensor("idxw", [P, K * NCHUNK * (NI // 16)], _I16,
                           kind="ExternalInput")
    out_d = nc.dram_tensor("out", [NGRP, P, NPAIR, GW], _F16,
                           kind="ExternalOutput")

    w_dram = nc.inline_tensor(w_pack, name="w_pack")
    b_dram = nc.inline_tensor(bias2, name="bias2")

    call_no = 0
    with tile.TileContext(nc) as tc:
        with (
            tc.tile_pool(name="consts", bufs=1) as cpool,
            tc.tile_pool(name="gp", bufs=12) as gpool,
            tc.tile_pool(name="op", bufs=2) as opool,
            tc.tile_pool(name="ps", bufs=8, space="PSUM") as pspool,
        ):
            # idx table first: gather descriptor prep only waits on this
            idx_sb = cpool.tile([P, K * NCHUNK * (NI // 16)], _I16)
            nc.sync.dma_start(idx_sb[:], idx_d[:])
            w_sb = cpool.tile([P, K + 1, P], _F16)
            nc.sync.dma_start(w_sb[:], w_dram[:])
            b_sb = cpool.tile([P, 1], _F32)
            nc.sync.dma_start(b_sb[:], b_dram[:])
            # whole center operand resident: no per-chunk load dependency
            x_sb = cpool.tile([P, NCHUNK, NPAIR, NI], _F8)
            nc.sync.dma_start(x_sb[:], xall_d[:])

            for g in range(NGRP):
                o_t = opool.tile([P, NPAIR, GW], _F16, name="o_t", tag="o_t")
                for cl in range(GRP):
                    ci = g * GRP + cl
                    g_ts = []
                    for k in range(K):
                        g_t = gpool.tile([P, NQUAD, NI], _F16)
                        io = (k * NCHUNK + ci) * (NI // 16)
                        nc.gpsimd.dma_gather(
                            g_t[:], xtab_d[:], idx_sb[:, io:io + NI // 16],
                            num_idxs=NI, num_idxs_reg=LIVE,
                            elem_size=ELEM, transpose=True,
                            queue_num=call_no % NQ)
                        call_no += 1
                        # fp8 view [p, q, m, idx]: pair index = 2q + m
                        g_ts.append(g_t[:].bitcast(_F8).rearrange(
                            "p q (i m) -> p q m i", m=2))
                    pss = [pspool.tile([P, 4, LIVE], _F32, name="ps",
                                       tag="ps")
                           for qd in range(NPAIR // 4)]
                    for qd in range(NPAIR // 4):
                        nc.tensor.matmul(
                            pss[qd][:, :, :], w_sb[:, 0, :],
                            x_sb[:, ci, qd * 4:qd * 4 + 4, :LIVE],
                            start=True, stop=False)
                    # tap-outer: each tap's matmuls only wait on its gather
                    for k in range(K):
                        for qd in range(NPAIR // 4):
                            nc.tensor.matmul(
                                pss[qd][:, :, :], w_sb[:, k + 1, :],
                                g_ts[k][:, qd * 2:qd * 2 + 2, :, :LIVE],
                                start=False, stop=(k == K - 1))
                    for qd in range(NPAIR // 4):
                        nc.vector.tensor_scalar_add(
                            o_t[:, qd * 4:qd * 4 + 4,
                                cl * LIVE:cl * LIVE + LIVE],
                            pss[qd][:, :, :], b_sb[:, :1])
                nc.sync.dma_start(out_d[g], o_t[:])

    nc.compile()
    return nc


def _run(inputs, trace=False):
    xtab, x_all, idx_w, w_pack, bias2 = _host_prep(
        inputs["x"], inputs["neighbors"], inputs["weight_center"],
        inputs["weight_neighbors"], inputs["bias"])
    nc = _build_program(w_pack, bias2)
    in_maps = []
    for bi in range(NB):
        for hj in range(NH):
            core = bi * NH + hj
            in_maps.append({"xtab": xtab[bi], "xall": x_all[core],
                            "idxw": idx_w[hj]})
    res = None
    for attempt in range(3):
        try:
            res = bass_utils.run_bass_kernel_spmd(
                nc, in_maps, core_ids=list(range(NCORES)), trace=trace)
            break
        except Exception:
            # transient NRT/device hiccups: retry (recompiles nothing)
            if attempt == 2:
                raise
    out = np.zeros((B, C, H), dtype=np.float32)
    for bi in range(NB):
        for hj in range(NH):
            core = bi * NH + hj
            r = np.asarray(res.results[core]["out"])  # [NGRP,128,NPAIR,GW]
            r = r.reshape(NGRP, 2, C, NPAIR, GW).astype(np.float32)
            r = r.transpose(3, 1, 2, 0, 4).reshape(BL, C, HHALF)
            out[bi * BL:(bi + 1) * BL, :, H0[hj]:H0[hj] + HHALF] = r
    return np.ascontiguousarray(out), res


def kernel(x, neighbors, weight_center, weight_neighbors, bias):
    out, _ = _run(dict(x=x, neighbors=neighbors, weight_center=weight_center,
                       weight_neighbors=weight_neighbors, bias=bias))
    return out
